# revision 5
# baseline (speedup 1.0000x reference)
"""Trainium2 Bass kernel for nn_MinkUNet (sparse voxel UNet stem + residual block).

Fully fused single-launch version: one bass module runs all stages on 8 cores.
  1. vox    : per-voxel mean of point features (batched indirect gather + matmul)
  2. conv1  : sparse 3x3x3 conv (Cin=4) + BN + ReLU
  3. conv2  : sparse 3x3x3 conv (Cin=32) + BN + ReLU
  4. r1     : sparse conv + BN + ReLU
  5. r2     : sparse conv + BN + residual + ReLU, fused classifier Y = h3 @ Wc
  6. devox  : trilinear 8-neighbor gather of Y + bias

Sharding: voxels/points split across 8 cores. After each stage the per-core
shard of the activation table is AllGather'd on device so every core holds the
full table for the next stage's random-access gathers. BN statistics are
AllReduce'd. All gathers are batched: one indirect DMA per supertile
(512 rows x K neighbors) instead of one per 128 rows.
"""
import numpy as np

import concourse.bass as bass
import concourse.mybir as mybir
from concourse.tile import TileContext
from concourse.masks import make_identity

f32 = mybir.dt.float32
bf16 = mybir.dt.bfloat16
i32 = mybir.dt.int32
ACT = mybir.ActivationFunctionType
ALU = mybir.AluOpType

# problem sizes (hardcoded per contract)
N, M, K, KD = 400000, 300000, 27, 8
CIN, C0, NCLS = 4, 32, 19
EPS = 1e-5
NC = 8
Ms = M // NC                      # 37500
MsP = 296 * 128                   # 37888 = 74*512
MT = NC * MsP                     # 303104
Np = N // NC                      # 50000
NpP = 392 * 128                   # 50176 = 98*512
ZR = Ms                           # zero row (shard-0 pad row 0) in padded coords
SUP = 4                           # tiles per supertile
NSUP_V = MsP // (SUP * 128)       # 74
NSUP_P = NpP // (SUP * 128)       # 98
RG = [list(range(NC))]

_cache = {}
LAUNCH_TIMES = []


# ---------------------------------------------------------------- wait splitting
def _split_sync_waits(bir_bytes, wait_limit=1):
    """Pinned walrus encodes at most 1 sync wait per instruction; split extras
    onto same-engine reg-move nops placed immediately before (same program
    order on the engine, semantically identical)."""
    import json
    m = json.loads(bir_bytes)
    ctr = [0]

    def nop(engine, on_wait):
        ctr[0] += 1
        return {
            "debug": 0, "engine": engine,
            "ins": [{"dtype": "int32", "kind": "imm_value", "value": 0}],
            "outs": [{"dtype": "int32", "kind": "register_access",
                      "regref": f"{engine}_zero"}],
            "name": f"wsplit-{ctr[0]}", "opcode": "RegisterMove",
            "sync_info": {"on_wait": on_wait, "on_update": []},
        }

    for f in m["functions"]:
        for b in f["blocks"]:
            out = []
            for ins in b["instructions"]:
                si = ins.get("sync_info")
                if si:
                    ow = si.get("on_wait") or []
                    if len(ow) > wait_limit:
                        extra, keep = ow[:-wait_limit], ow[-wait_limit:]
                        for i in range(0, len(extra), wait_limit):
                            out.append(nop(ins["engine"], extra[i:i + wait_limit]))
                        si["on_wait"] = keep
                out.append(ins)
            b["instructions"] = out
    return json.dumps(m).encode()


def _install_waitfix(nc):
    orig = nc.to_json_bytes
    nc.to_json_bytes = lambda: _split_sync_waits(orig())
    return nc


# ---------------------------------------------------------------- SPMD runner
class _Runner:
    """jit once; inputs device_put per call; mirrors bass2jax multi-core path."""

    def __init__(self, nc):
        import jax
        from jax.sharding import Mesh, PartitionSpec, NamedSharding
        from jax.experimental.shard_map import shard_map
        from concourse import bass2jax
        from concourse.bass2jax import _bass_exec_p, install_neuronx_cc_hook
        install_neuronx_cc_hook()
        self.jax = jax
        self.nc = nc
        pname = nc.partition_id_tensor.name if nc.partition_id_tensor else None
        in_names, out_names, out_avals, zero_shapes = [], [], [], []
        for alloc in nc.m.functions[0].allocations:
            if not isinstance(alloc, mybir.MemoryLocationSet):
                continue
            name = alloc.memorylocations[0].name
            if alloc.kind == "ExternalInput":
                if name != pname:
                    in_names.append(name)
            elif alloc.kind == "ExternalOutput":
                out_names.append(name)
                shape = tuple(alloc.tensor_shape)
                dtype = mybir.dt.np(alloc.dtype)
                out_avals.append(jax.core.ShapedArray(shape, dtype))
                zero_shapes.append((shape, dtype))
        self.in_names, self.out_names, self.out_avals = in_names, out_names, out_avals
        all_in = list(in_names) + list(out_names)
        if pname is not None:
            all_in.append(pname)
        n_params, n_outs = len(in_names), len(out_names)

        def _body(*args):
            operands = list(args)
            if pname is not None:
                operands.append(bass2jax.partition_id_tensor())
            return tuple(_bass_exec_p.bind(
                *operands, out_avals=tuple(out_avals), in_names=tuple(all_in),
                out_names=tuple(out_names), lowering_input_output_aliases=(),
                sim_require_finite=True, sim_require_nnan=True, nc=nc))

        devices = jax.devices()[:NC]
        self.mesh = Mesh(np.asarray(devices), ("core",))
        specs_in = (PartitionSpec("core"),) * (n_params + n_outs)
        specs_out = (PartitionSpec("core"),) * n_outs
        self.fn = jax.jit(
            shard_map(_body, mesh=self.mesh, in_specs=specs_in,
                      out_specs=specs_out, check_rep=False),
            keep_unused=True)
        self.sharding = NamedSharding(self.mesh, PartitionSpec("core"))
        self.zeros = [
            self.jax.device_put(
                np.zeros((NC * s[0], *s[1:]), d), self.sharding)
            for s, d in zero_shapes
        ]

    def __call__(self, in_maps):
        concat = [
            np.concatenate([np.asarray(in_maps[c][n]) for c in range(NC)], 0)
            for n in self.in_names
        ]
        args = [self.jax.device_put(a, self.sharding) for a in concat]
        self.jax.block_until_ready(args)
        import time as _time
        _t0 = _time.perf_counter()
        outs = self.fn(*args, *self.zeros)
        self.jax.block_until_ready(outs)
        LAUNCH_TIMES.append(_time.perf_counter() - _t0)
        res = []
        for c in range(NC):
            res.append({
                n: np.asarray(outs[i]).reshape(NC, *self.out_avals[i].shape)[c]
                for i, n in enumerate(self.out_names)
            })
        return res


# ---------------------------------------------------------------- module builder
def _bn_affine(nc, pool, st, g_sb, b_sb, ci):
    """st [32,2] (sum, sumsq over M) -> (a, bb) [32,1] tiles."""
    mean = pool.tile([32, 1], f32, name=f"bn_mean{ci}")
    ex2 = pool.tile([32, 1], f32, name=f"bn_ex2{ci}")
    nc.vector.tensor_scalar_mul(mean[:], st[:, 0:1], 1.0 / M)
    nc.vector.tensor_scalar_mul(ex2[:], st[:, 1:2], 1.0 / M)
    m2 = pool.tile([32, 1], f32, name=f"bn_m2{ci}")
    nc.vector.tensor_tensor(out=m2[:], in0=mean[:], in1=mean[:], op=ALU.mult)
    var = pool.tile([32, 1], f32, name=f"bn_var{ci}")
    nc.vector.tensor_tensor(out=var[:], in0=ex2[:], in1=m2[:], op=ALU.subtract)
    vp = pool.tile([32, 1], f32, name=f"bn_vp{ci}")
    nc.vector.tensor_scalar_add(vp[:], var[:], EPS)
    std = pool.tile([32, 1], f32, name=f"bn_std{ci}")
    nc.scalar.activation(out=std[:], in_=vp[:], func=ACT.Sqrt)
    inv = pool.tile([32, 1], f32, name=f"bn_inv{ci}")
    nc.vector.reciprocal(inv[:], std[:])
    a = pool.tile([32, 1], f32, name=f"bn_a{ci}")
    nc.vector.tensor_tensor(out=a[:], in0=g_sb[:], in1=inv[:], op=ALU.mult)
    ma = pool.tile([32, 1], f32, name=f"bn_ma{ci}")
    nc.vector.tensor_tensor(out=ma[:], in0=mean[:], in1=a[:], op=ALU.mult)
    bb = pool.tile([32, 1], f32, name=f"bn_bb{ci}")
    nc.vector.tensor_tensor(out=bb[:], in0=b_sb[:], in1=ma[:], op=ALU.subtract)
    return a, bb


DEBUG = False
DEBUG_RESULTS = None
PR = 64
_gq = [0]


def _gather(nc, out_ap, table_ap, idx_col, nq=4):
    """One indirect DMA: 128 rows (one index per partition) — the only
    batching the SWDGE ucode supports (one index per partition per instr)."""
    inst = nc.gpsimd.indirect_dma_start(
        out=out_ap, out_offset=None, in_=table_ap,
        in_offset=bass.IndirectOffsetOnAxis(ap=idx_col, axis=0))
    q = _gq[0] % nq
    _gq[0] += 1
    if q:
        inst.ins.queue = f"qPoolDynamic{q}"


def build_fused(dmax, debug=False):
    nc = bass.Bass(num_swdge_queues=4)
    # ---- parameters (per core)
    pf = nc.declare_dram_parameter("pf", [N + dmax, CIN], f32, isOutput=False)
    vstart = nc.declare_dram_parameter("vstart", [MsP, 1], i32, isOutput=False)
    vmask = nc.declare_dram_parameter("vmask", [MsP, dmax], f32, isOutput=False)
    rcp = nc.declare_dram_parameter("rcp", [MsP, 1], f32, isOutput=False)
    smat = nc.declare_dram_parameter("smat", [dmax * CIN, CIN], f32, isOutput=False)
    nbrs = nc.declare_dram_parameter("nbrs", [MsP, K], i32, isOutput=False)
    didx = nc.declare_dram_parameter("didx", [NpP, KD], i32, isOutput=False)
    wdev = nc.declare_dram_parameter("wdev", [NpP, KD], f32, isOutput=False)
    w1 = nc.declare_dram_parameter("w1", [128, C0], f32, isOutput=False)
    w2 = nc.declare_dram_parameter("w2", [896, C0], bf16, isOutput=False)
    wr1 = nc.declare_dram_parameter("wr1", [896, C0], bf16, isOutput=False)
    wr2 = nc.declare_dram_parameter("wr2", [896, C0], bf16, isOutput=False)
    gps = [nc.declare_dram_parameter(f"gp{i}", [C0], f32, isOutput=False)
           for i in range(4)]
    bps = [nc.declare_dram_parameter(f"bp{i}", [C0], f32, isOutput=False)
           for i in range(4)]
    wc = nc.declare_dram_parameter("wc", [C0, C0], f32, isOutput=False)
    bc = nc.declare_dram_parameter("bc", [1, C0], f32, isOutput=False)
    out = nc.declare_dram_parameter("out", [NpP, NCLS], f32, isOutput=True)
    dbg = {}
    if debug:
        for nm, w in [("vox", CIN), ("h1", C0), ("h2", C0), ("r1", C0),
                      ("y", C0)]:
            dbg[nm] = nc.declare_dram_parameter(f"dbg_{nm}", [NC * PR, w], f32,
                                                isOutput=True)
        dbg["rawT0"] = nc.declare_dram_parameter("dbg_rawT0", [32, PR], f32,
                                                 isOutput=True)

    # ---- internal DRAM
    vox_sh = nc.dram_tensor("vox_sh", [MsP, CIN], f32)
    vox_full = nc.dram_tensor("vox_full", [MT, CIN], f32, addr_space="Shared")
    h_sh = [nc.dram_tensor(f"h_sh{i}", [MsP, C0], bf16) for i in range(4)]
    h_full = [nc.dram_tensor(f"h_full{i}", [MT, C0], bf16, addr_space="Shared")
              for i in range(4)]
    rawT = [nc.dram_tensor(f"rawT{i}", [32, MsP], f32) for i in range(4)]
    st_in = [nc.dram_tensor(f"st_in{i}", [32, 2], f32) for i in range(4)]
    st_out = [nc.dram_tensor(f"st_out{i}", [32, 2], f32, addr_space="Shared")
              for i in range(4)]

    conv_ws = [w1, w2, wr1, wr2]
    conv_cin = [CIN, C0, C0, C0]
    conv_tab = [vox_full, h_full[0], h_full[1], h_full[2]]

    with TileContext(nc) as tc:
        with tc.tile_pool(name="const", bufs=1) as cp:
            ident = cp.tile([128, 128], f32, name="ident")
            make_identity(nc, ident[:])
            identb = cp.tile([128, 128], bf16, name="identb")
            make_identity(nc, identb[:])
            zt = cp.tile([97, 128], bf16, name="zt")
            nc.vector.memset(zt[:], 0.0)
            zb = cp.tile([128, 128], bf16, name="zb")
            nc.vector.memset(zb[:], 0.0)

            # ================= stage 1: voxelize =================
            GWv = dmax * CIN
            with (
                tc.tile_pool(name="sbV", bufs=3) as sb,
                tc.tile_pool(name="ppV", bufs=2, space="PSUM") as pp,
            ):
                ssb = cp.tile([GWv, CIN], f32, name="ssb")
                nc.sync.dma_start(out=ssb[:], in_=smat[:])
                vst_r = vstart[:].rearrange("(s t p) o -> s p t o", t=SUP, p=128)
                vmk_r = vmask[:].rearrange("(s t p) k -> s p t k", t=SUP, p=128)
                rcp_r = rcp[:].rearrange("(s t p) o -> s p t o", t=SUP, p=128)
                vout_r = vox_sh[:].rearrange("(s t p) c -> s p t c", t=SUP, p=128)
                for s in range(NSUP_V):
                    idx = sb.tile([128, SUP], i32, name="idxV", tag="idxV")
                    nc.sync.dma_start(
                        out=idx[:].rearrange("p (t o) -> p t o", t=SUP),
                        in_=vst_r[s])
                    # points are sorted by voxel: one indexed contiguous run of
                    # dmax point rows per voxel (one gather per 128 voxels)
                    G = sb.tile([128, SUP * GWv], f32, name="GV", tag="GV")
                    for t in range(SUP):
                        _gather(nc, G[:, t * GWv:(t + 1) * GWv],
                                pf[:], idx[:, t:t + 1])
                    mk = sb.tile([128, SUP * dmax], f32, name="mkV", tag="mkV")
                    nc.sync.dma_start(
                        out=mk[:].rearrange("p (t k) -> p t k", t=SUP),
                        in_=vmk_r[s])
                    Gm = sb.tile([128, SUP * GWv], f32, name="GmV", tag="GmV")
                    mkv = mk[:].rearrange("p (t d) -> p t d", t=SUP)
                    mkb = bass.AP(mkv.tensor, mkv.offset,
                                  [list(mkv.ap[0]), list(mkv.ap[1]),
                                   list(mkv.ap[2]), [0, CIN]])
                    nc.vector.tensor_tensor(
                        out=Gm[:].rearrange("p (t d c) -> p t d c", t=SUP, d=dmax),
                        in0=G[:].rearrange("p (t d c) -> p t d c", t=SUP, d=dmax),
                        in1=mkb, op=ALU.mult)
                    pgt = pp.tile([128, 512], f32, name="pgtV", tag="pgtV")
                    for t in range(SUP):
                        nc.tensor.transpose(out=pgt[:GWv, t * 128:(t + 1) * 128],
                                            in_=Gm[:, t * GWv:(t + 1) * GWv],
                                            identity=ident[:])
                    GT = sb.tile([128, 512], f32, name="GTV", tag="GTV")
                    nc.vector.tensor_copy(out=GT[:GWv, :], in_=pgt[:GWv, :])
                    pv = pp.tile([128, SUP * CIN], f32, name="pvV", tag="pvV")
                    for t in range(SUP):
                        nc.tensor.matmul(out=pv[:, t * CIN:(t + 1) * CIN],
                                         lhsT=GT[:GWv, t * 128:(t + 1) * 128],
                                         rhs=ssb[:], start=True, stop=True)
                    rc = sb.tile([128, SUP], f32, name="rcV", tag="rcV")
                    nc.sync.dma_start(
                        out=rc[:].rearrange("p (t o) -> p t o", t=SUP),
                        in_=rcp_r[s])
                    vsb = sb.tile([128, SUP * CIN], f32, name="vsbV", tag="vsbV")
                    rcb = bass.AP(rc[:].tensor, rc[:].offset,
                                  [list(rc[:].ap[0]), [1, SUP], [0, CIN]])
                    nc.vector.tensor_tensor(
                        out=vsb[:].rearrange("p (t c) -> p t c", t=SUP),
                        in0=pv[:].rearrange("p (t c) -> p t c", t=SUP),
                        in1=rcb, op=ALU.mult)
                    nc.sync.dma_start(out=vout_r[s],
                                      in_=vsb[:].rearrange("p (t c) -> p t c",
                                                           t=SUP))
            nc.gpsimd.collective_compute("AllGather", ALU.bypass, RG,
                                         ins=[vox_sh[:]], outs=[vox_full[:]])

            # ================= stages 2-5: conv layers =================
            nbrs_r = nbrs[:].rearrange("(s t p) k -> s p t k", t=SUP, p=128)
            for ci in range(4):
                cc = conv_cin[ci]
                GW = K * cc
                nchunk = (GW + 127) // 128
                table = conv_tab[ci]
                residual = (ci == 3)
                tdt = f32 if ci == 0 else bf16      # gather-path dtype
                tid = ident if ci == 0 else identb
                # ---- pass A: raw conv -> rawT + stats
                with (
                    tc.tile_pool(name=f"sbA{ci}", bufs=4) as sb,
                    tc.tile_pool(name=f"ppA{ci}", bufs=2, space="PSUM") as pp,
                ):
                    wsb = cp.tile([128, nchunk * C0], tdt, name=f"wsb{ci}")
                    nc.sync.dma_start(
                        out=wsb[:].rearrange("p (j c) -> p j c", j=nchunk),
                        in_=conv_ws[ci][:].rearrange("(j p) c -> p j c", p=128))
                    sums = cp.tile([32, NSUP_V], f32, name=f"sums{ci}")
                    sqs = cp.tile([32, NSUP_V], f32, name=f"sqs{ci}")
                    for s in range(NSUP_V):
                        idx = sb.tile([128, SUP * K], i32, name="idxA", tag="idxA")
                        nc.sync.dma_start(
                            out=idx[:].rearrange("p (t k) -> p t k", t=SUP),
                            in_=nbrs_r[s])
                        G = sb.tile([128, SUP * GW], tdt, name="GA", tag="GA")
                        for t in range(SUP):
                            for k in range(K):
                                _gather(nc,
                                        G[:, t * GW + k * cc: t * GW + (k + 1) * cc],
                                        table[:], idx[:, t * K + k: t * K + k + 1])
                        po = pp.tile([32, 512], f32, name="poA", tag="poA")
                        for j in range(nchunk):
                            pgt = pp.tile([128, 512], tdt, name="pgtA", tag="pgtA")
                            cw = min(128, GW - j * 128)
                            if cw < 128:
                                if tdt == f32:
                                    nc.vector.memset(pgt[:], 0.0)
                                else:
                                    # DVE can't memset bf16 PSUM; zero via PE
                                    for t in range(SUP):
                                        nc.tensor.transpose(
                                            out=pgt[:, t * 128:(t + 1) * 128],
                                            in_=zb[:], identity=identb[:])
                            for t in range(SUP):
                                nc.tensor.transpose(
                                    out=pgt[:cw, t * 128:(t + 1) * 128],
                                    in_=G[:, t * GW + j * 128: t * GW + j * 128 + cw],
                                    identity=tid[:])
                            GT = sb.tile([128, 512], tdt, name="GTA", tag="GTA")
                            if j % 2 == 0:
                                nc.vector.tensor_copy(out=GT[:], in_=pgt[:])
                            else:
                                nc.scalar.activation(out=GT[:], in_=pgt[:],
                                                     func=ACT.Copy)
                            nc.tensor.matmul(out=po[:],
                                             lhsT=wsb[:, j * C0:(j + 1) * C0],
                                             rhs=GT[:], start=(j == 0),
                                             stop=(j == nchunk - 1))
                        rawsb = sb.tile([32, 512], f32, name="rawA", tag="rawA")
                        nc.scalar.activation(out=rawsb[:], in_=po[:], func=ACT.Copy,
                                             accum_out=sums[:, s:s + 1])
                        sqsb = sb.tile([32, 512], f32, name="sqA", tag="sqA")
                        nc.vector.tensor_tensor(out=sqsb[:], in0=rawsb[:],
                                                in1=rawsb[:], op=ALU.mult)
                        nc.vector.tensor_reduce(out=sqs[:, s:s + 1], in_=sqsb[:],
                                                axis=mybir.AxisListType.X,
                                                op=ALU.add)
                        nc.sync.dma_start(out=rawT[ci][:, s * 512:(s + 1) * 512],
                                          in_=rawsb[:])
                    stats = cp.tile([32, 2], f32, name=f"stats{ci}")
                    nc.vector.tensor_reduce(out=stats[:, 0:1], in_=sums[:],
                                            axis=mybir.AxisListType.X, op=ALU.add)
                    nc.vector.tensor_reduce(out=stats[:, 1:2], in_=sqs[:],
                                            axis=mybir.AxisListType.X, op=ALU.add)
                    nc.sync.dma_start(out=st_in[ci][:], in_=stats[:])
                    nc.gpsimd.collective_compute("AllReduce", ALU.add, RG,
                                                 ins=[st_in[ci][:]],
                                                 outs=[st_out[ci][:]])
                    star = cp.tile([32, 2], f32, name=f"star{ci}")
                    nc.sync.dma_start(out=star[:], in_=st_out[ci][:])
                    gsb = cp.tile([32, 1], f32, name=f"gsb{ci}")
                    bsb = cp.tile([32, 1], f32, name=f"bsb{ci}")
                    nc.sync.dma_start(out=gsb[:], in_=gps[ci][:, None])
                    nc.sync.dma_start(out=bsb[:], in_=bps[ci][:, None])
                    a, bb = _bn_affine(nc, cp, star, gsb, bsb, ci)

                # ---- pass B: BN affine (+ residual / classifier) -> h_sh[ci]
                hout = h_sh[ci]
                with (
                    tc.tile_pool(name=f"sbB{ci}", bufs=3) as sb,
                    tc.tile_pool(name=f"ppB{ci}", bufs=2, space="PSUM") as pp,
                ):
                    if residual:
                        wcsb = cp.tile([C0, C0], f32, name="wcsb")
                        nc.sync.dma_start(out=wcsb[:], in_=wc[:])
                        h2_r = h_sh[1][:].rearrange("(s t p) c -> s p t c",
                                                    t=SUP, p=128)
                    hout_r = hout[:].rearrange("(s t p) c -> s p t c", t=SUP, p=128)
                    for s in range(NSUP_V):
                        raw2 = sb.tile([32, 512], f32, name="raw2", tag="raw2")
                        nc.sync.dma_start(out=raw2[:],
                                          in_=rawT[ci][:, s * 512:(s + 1) * 512])
                        if not residual:
                            hT = sb.tile([32, 512], f32, name="hT", tag="hT")
                            nc.scalar.activation(out=hT[:], in_=raw2[:],
                                                 func=ACT.Relu, bias=bb[:],
                                                 scale=a[:])
                            ph = pp.tile([128, 128], f32, name="ph", tag="ph")
                            for t in range(SUP):
                                nc.tensor.transpose(
                                    out=ph[:, t * C0:(t + 1) * C0],
                                    in_=hT[:, t * 128:(t + 1) * 128],
                                    identity=ident[:32, :32])
                            hsb = sb.tile([128, 128], bf16, name="hsb", tag="hsb")
                            nc.vector.tensor_copy(out=hsb[:], in_=ph[:])
                            nc.sync.dma_start(
                                out=hout_r[s],
                                in_=hsb[:].rearrange("p (t c) -> p t c", t=SUP))
                        else:
                            t0 = sb.tile([32, 512], f32, name="t0", tag="t0")
                            nc.scalar.activation(out=t0[:], in_=raw2[:],
                                                 func=ACT.Identity, bias=bb[:],
                                                 scale=a[:])
                            h2sb = sb.tile([128, 128], f32, name="h2sb", tag="h2sb")
                            nc.gpsimd.dma_start(
                                out=h2sb[:].rearrange("p (t c) -> p t c", t=SUP),
                                in_=h2_r[s])
                            ph2 = pp.tile([32, 512], f32, name="ph2", tag="ph2")
                            for t in range(SUP):
                                nc.tensor.transpose(
                                    out=ph2[:, t * 128:(t + 1) * 128],
                                    in_=h2sb[:, t * C0:(t + 1) * C0],
                                    identity=ident[:])
                            s1 = sb.tile([32, 512], f32, name="s1", tag="s1")
                            nc.vector.tensor_tensor(out=s1[:], in0=t0[:],
                                                    in1=ph2[:], op=ALU.add)
                            h3 = sb.tile([32, 512], f32, name="h3", tag="h3")
                            nc.vector.tensor_scalar_max(h3[:], s1[:], 0.0)
                            py = pp.tile([128, 128], f32, name="py", tag="py")
                            for t in range(SUP):
                                nc.tensor.matmul(
                                    out=py[:, t * C0:(t + 1) * C0],
                                    lhsT=h3[:, t * 128:(t + 1) * 128],
                                    rhs=wcsb[:], start=True, stop=True)
                            ysb = sb.tile([128, 128], bf16, name="ysb", tag="ysb")
                            nc.vector.tensor_copy(out=ysb[:], in_=py[:])
                            nc.sync.dma_start(
                                out=hout_r[s],
                                in_=ysb[:].rearrange("p (t c) -> p t c", t=SUP))
                    # zero the shard's pad rows (Ms..MsP) so gathers of pad
                    # indices and the ZR row read zeros
                    nc.sync.dma_start(
                        out=hout[Ms:MsP].rearrange("(p r) c -> p r c", p=97),
                        in_=zt[:].rearrange("p (r c) -> p r c", r=4))
                nc.gpsimd.collective_compute("AllGather", ALU.bypass, RG,
                                             ins=[hout[:]], outs=[h_full[ci][:]])

            # ================= stage 6: devoxelize =================
            ytab = h_full[3]
            with (
                tc.tile_pool(name="sbD", bufs=3) as sb,
                tc.tile_pool(name="ppD", bufs=2, space="PSUM") as pp,
            ):
                ones = cp.tile([1, 128], f32, name="onesD")
                nc.gpsimd.memset(ones[:], 1.0)
                bcs = cp.tile([1, C0], f32, name="bcsD")
                nc.sync.dma_start(out=bcs[:], in_=bc[:])
                pbc = pp.tile([128, C0], f32, name="pbc")
                nc.tensor.matmul(out=pbc[:], lhsT=ones[:], rhs=bcs[:],
                                 start=True, stop=True)
                bcb = cp.tile([128, C0], f32, name="bcbD")
                nc.vector.tensor_copy(out=bcb[:], in_=pbc[:])

                didx_r = didx[:].rearrange("(s t p) k -> s p t k", t=SUP, p=128)
                wdev_r = wdev[:].rearrange("(s t p) k -> s p t k", t=SUP, p=128)
                out_r = out[:].rearrange("(s t p) c -> s p t c", t=SUP, p=128)
                GW = KD * C0
                for s in range(NSUP_P):
                    idx = sb.tile([128, SUP * KD], i32, name="idxD", tag="idxD")
                    nc.sync.dma_start(
                        out=idx[:].rearrange("p (t k) -> p t k", t=SUP),
                        in_=didx_r[s])

                    G = sb.tile([128, SUP * GW], bf16, name="GD", tag="GD")
                    for t in range(SUP):
                        for k in range(KD):
                            _gather(nc,
                                    G[:, t * GW + k * C0: t * GW + (k + 1) * C0],
                                    ytab[:], idx[:, t * KD + k: t * KD + k + 1])
                    Gf = sb.tile([128, SUP * GW], f32, name="GfD", tag="GfD")
                    nc.scalar.activation(out=Gf[:], in_=G[:], func=ACT.Copy)
                    w4 = sb.tile([128, SUP * KD], f32, name="w4", tag="w4")
                    nc.sync.dma_start(
                        out=w4[:].rearrange("p (t k) -> p t k", t=SUP),
                        in_=wdev_r[s])
                    prod = sb.tile([128, SUP * GW], f32, name="prod", tag="prod")
                    gv = Gf[:].rearrange("p (t k c) -> p t k c", t=SUP, k=KD, c=C0)
                    pvw = prod[:].rearrange("p (t c k) -> p t k c", t=SUP, c=C0,
                                            k=KD)
                    wv = w4[:].rearrange("p (t k) -> p t k", t=SUP)
                    wb = bass.AP(wv.tensor, wv.offset,
                                 [list(wv.ap[0]), list(wv.ap[1]), list(wv.ap[2]),
                                  [0, C0]])
                    nc.vector.tensor_tensor(out=pvw, in0=gv, in1=wb, op=ALU.mult)
                    pts = sb.tile([128, SUP * C0], f32, name="pts", tag="pts")
                    nc.vector.tensor_reduce(
                        out=pts[:].rearrange("p (t c) -> p t c", t=SUP),
                        in_=prod[:].rearrange("p (t c k) -> p t c k", t=SUP,
                                              c=C0, k=KD),
                        axis=mybir.AxisListType.X, op=ALU.add)
                    res = sb.tile([128, SUP * C0], f32, name="res", tag="res")
                    bcv = bass.AP(bcb[:].tensor, bcb[:].offset,
                                  [list(bcb[:].ap[0]), [0, SUP],
                                   list(bcb[:].ap[1])])
                    nc.vector.tensor_tensor(
                        out=res[:].rearrange("p (t c) -> p t c", t=SUP),
                        in0=pts[:].rearrange("p (t c) -> p t c", t=SUP),
                        in1=bcv, op=ALU.add)
                    nc.sync.dma_start(
                        out=out_r[s],
                        in_=res[:].rearrange("p (t c) -> p t c", t=SUP)[:, :, :NCLS])

            if debug:
                with tc.tile_pool(name="sbDbg", bufs=2) as sb:
                    tabs = dict(vox=vox_full, h1=h_full[0], h2=h_full[1],
                                r1=h_full[2], y=h_full[3])
                    for nm, tab in tabs.items():
                        w = CIN if nm == "vox" else C0
                        for r in range(NC):
                            t = sb.tile([PR, w], f32, name=f"dbg_{nm}{r}",
                                        tag="dbgt")
                            nc.sync.dma_start(
                                out=t[:], in_=tab[r * MsP:r * MsP + PR])
                            nc.sync.dma_start(
                                out=dbg[nm][r * PR:(r + 1) * PR], in_=t[:])
                    t = sb.tile([32, PR], f32, name="dbg_rawT0", tag="dbgr")
                    nc.sync.dma_start(out=t[:], in_=rawT[0][:, :PR])
                    nc.sync.dma_start(out=dbg["rawT0"][:], in_=t[:])
    return _install_waitfix(nc)


# ---------------------------------------------------------------- host side
def _remap(g):
    g = np.asarray(g)
    gc = np.clip(g, 0, M - 1)
    s = gc // Ms
    out = s * MsP + (gc - s * Ms)
    return np.where(g < 0, ZR, out).astype(np.int32)


def _stack_w(Wk, cols):
    """W [27, cin, 32] -> padded [nchunk*128, 32] stack over (k, cin)."""
    Wk = np.asarray(Wk, np.float32)
    kcin = Wk.shape[0] * Wk.shape[1]
    nchunk = (27 * Wk.shape[1] + 127) // 128
    o = np.zeros((nchunk * 128, C0), np.float32)
    o[:kcin] = Wk.reshape(kcin, C0)
    return o


def _get_runner(dmax):
    key = ("fused", dmax, DEBUG)
    if key not in _cache:
        _cache[key] = _Runner(build_fused(dmax, debug=DEBUG))
    return _cache[key]


def kernel(point_fea, idx_query, nbrs, idx_dev, w_dev,
           W_s1, W_s2, g_s1, b_s1, g_s2, b_s2,
           W_r1, W_r2, g_r1, b_r1, g_r2, b_r2, W_c, b_c):
    point_fea = np.asarray(point_fea, np.float32)
    idx_query = np.asarray(idx_query, np.int32)
    nbrs = np.asarray(nbrs, np.int32)
    idx_dev = np.asarray(idx_dev, np.int32)
    w_dev = np.asarray(w_dev, np.float32)

    # ---- host preprocessing (index plumbing only)
    counts = np.bincount(idx_query, minlength=M)
    dmax = int(counts.max())
    order = np.argsort(idx_query, kind="stable")
    starts = np.zeros(M + 1, np.int64)
    np.cumsum(counts, out=starts[1:])
    # points sorted by voxel id: each voxel's points are contiguous rows
    pf_table = np.zeros((N + dmax, CIN), np.float32)
    pf_table[:N] = point_fea[order]
    vstart_full = starts[:M].astype(np.int32)          # [M]
    vmask_full = (np.arange(dmax)[None, :]
                  < counts[:, None]).astype(np.float32)  # [M, dmax]
    recip_full = (1.0 / np.maximum(counts, 1)).astype(np.float32)

    smat = np.zeros((dmax * CIN, CIN), np.float32)
    for d in range(dmax):
        smat[d * CIN:(d + 1) * CIN] = np.eye(CIN, dtype=np.float32)

    nb_remap = _remap(nbrs)                     # [M, 27]
    per = []
    for c in range(NC):
        vs = slice(c * Ms, (c + 1) * Ms)
        ps = slice(c * Np, (c + 1) * Np)
        vstart = np.full((MsP, 1), N, np.int32)
        vstart[:Ms, 0] = vstart_full[vs]
        vmask = np.zeros((MsP, dmax), np.float32)
        vmask[:Ms] = vmask_full[vs]
        rcp = np.zeros((MsP, 1), np.float32)
        rcp[:Ms, 0] = recip_full[vs]
        nb28 = np.full((MsP, K), ZR, np.int32)
        nb28[:Ms] = nb_remap[vs]
        didx = np.full((NpP, KD), ZR, np.int32)
        didx[:Np] = _remap(idx_dev[ps])
        wd = np.zeros((NpP, KD), np.float32)
        wd[:Np] = w_dev[ps]
        per.append(dict(vstart=vstart, vmask=vmask, rcp=rcp, nb28=nb28,
                        didx=didx, wd=wd))

    import ml_dtypes
    BF = ml_dtypes.bfloat16
    W1s = _stack_w(np.asarray(W_s1), CIN)
    W2s = _stack_w(np.asarray(W_s2), C0).astype(BF)
    Wr1s = _stack_w(np.asarray(W_r1), C0).astype(BF)
    Wr2s = _stack_w(np.asarray(W_r2), C0).astype(BF)
    Wc_pad = np.zeros((C0, C0), np.float32)
    Wc_pad[:, :NCLS] = np.asarray(W_c)
    bc_pad = np.zeros((1, C0), np.float32)
    bc_pad[0, :NCLS] = np.asarray(b_c)

    R = _get_runner(dmax)
    res = R([dict(pf=pf_table, vstart=per[c]["vstart"], vmask=per[c]["vmask"],
                  rcp=per[c]["rcp"],
                  smat=smat, nbrs=per[c]["nb28"], didx=per[c]["didx"],
                  wdev=per[c]["wd"], w1=W1s, w2=W2s, wr1=Wr1s, wr2=Wr2s,
                  gp0=np.asarray(g_s1, np.float32),
                  bp0=np.asarray(b_s1, np.float32),
                  gp1=np.asarray(g_s2, np.float32),
                  bp1=np.asarray(b_s2, np.float32),
                  gp2=np.asarray(g_r1, np.float32),
                  bp2=np.asarray(b_r1, np.float32),
                  gp3=np.asarray(g_r2, np.float32),
                  bp3=np.asarray(b_r2, np.float32),
                  wc=Wc_pad, bc=bc_pad)
             for c in range(NC)])
    if DEBUG:
        global DEBUG_RESULTS
        DEBUG_RESULTS = res
    out = np.concatenate([res[c]["out"][:Np] for c in range(NC)], 0)
    return np.ascontiguousarray(out)


# revision 6
# speedup vs baseline: 1.1394x; 1.1394x over previous
"""Trainium2 Bass kernel for nn_MinkUNet (sparse voxel UNet stem + residual block).

Fully fused single-launch version: one bass module runs all stages on 8 cores.
  1. vox    : per-voxel mean of point features (batched indirect gather + matmul)
  2. conv1  : sparse 3x3x3 conv (Cin=4) + BN + ReLU
  3. conv2  : sparse 3x3x3 conv (Cin=32) + BN + ReLU
  4. r1     : sparse conv + BN + ReLU
  5. r2     : sparse conv + BN + residual + ReLU, fused classifier Y = h3 @ Wc
  6. devox  : trilinear 8-neighbor gather of Y + bias

Sharding: voxels/points split across 8 cores. After each stage the per-core
shard of the activation table is AllGather'd on device so every core holds the
full table for the next stage's random-access gathers. BN statistics are
AllReduce'd. All gathers are batched: one indirect DMA per supertile
(512 rows x K neighbors) instead of one per 128 rows.
"""
import numpy as np

import concourse.bass as bass
import concourse.mybir as mybir
from concourse.tile import TileContext
from concourse.masks import make_identity

f32 = mybir.dt.float32
bf16 = mybir.dt.bfloat16
i32 = mybir.dt.int32
ACT = mybir.ActivationFunctionType
ALU = mybir.AluOpType

# problem sizes (hardcoded per contract)
N, M, K, KD = 400000, 300000, 27, 8
CIN, C0, NCLS = 4, 32, 19
EPS = 1e-5
NC = 8
Ms = M // NC                      # 37500
MsP = 296 * 128                   # 37888 = 74*512
MT = NC * MsP                     # 303104
Np = N // NC                      # 50000
NpP = 392 * 128                   # 50176 = 98*512
ZR = Ms                           # zero row (shard-0 pad row 0) in padded coords
SUP = 4                           # tiles per supertile
NSUP_V = MsP // (SUP * 128)       # 74
NSUP_P = NpP // (SUP * 128)       # 98
RG = [list(range(NC))]

_cache = {}
LAUNCH_TIMES = []


# ---------------------------------------------------------------- wait splitting
def _split_sync_waits(bir_bytes, wait_limit=1):
    """Pinned walrus encodes at most 1 sync wait per instruction; split extras
    onto same-engine reg-move nops placed immediately before (same program
    order on the engine, semantically identical)."""
    import json
    m = json.loads(bir_bytes)
    ctr = [0]

    def nop(engine, on_wait):
        ctr[0] += 1
        return {
            "debug": 0, "engine": engine,
            "ins": [{"dtype": "int32", "kind": "imm_value", "value": 0}],
            "outs": [{"dtype": "int32", "kind": "register_access",
                      "regref": f"{engine}_zero"}],
            "name": f"wsplit-{ctr[0]}", "opcode": "RegisterMove",
            "sync_info": {"on_wait": on_wait, "on_update": []},
        }

    for f in m["functions"]:
        for b in f["blocks"]:
            out = []
            for ins in b["instructions"]:
                si = ins.get("sync_info")
                if si:
                    ow = si.get("on_wait") or []
                    if len(ow) > wait_limit:
                        extra, keep = ow[:-wait_limit], ow[-wait_limit:]
                        for i in range(0, len(extra), wait_limit):
                            out.append(nop(ins["engine"], extra[i:i + wait_limit]))
                        si["on_wait"] = keep
                out.append(ins)
            b["instructions"] = out
    return json.dumps(m).encode()


def _install_waitfix(nc):
    orig = nc.to_json_bytes
    nc.to_json_bytes = lambda: _split_sync_waits(orig())
    return nc


# ---------------------------------------------------------------- SPMD runner
class _Runner:
    """jit once; inputs device_put per call; mirrors bass2jax multi-core path."""

    def __init__(self, nc):
        import jax
        from jax.sharding import Mesh, PartitionSpec, NamedSharding
        from jax.experimental.shard_map import shard_map
        from concourse import bass2jax
        from concourse.bass2jax import _bass_exec_p, install_neuronx_cc_hook
        install_neuronx_cc_hook()
        self.jax = jax
        self.nc = nc
        pname = nc.partition_id_tensor.name if nc.partition_id_tensor else None
        in_names, out_names, out_avals, zero_shapes = [], [], [], []
        for alloc in nc.m.functions[0].allocations:
            if not isinstance(alloc, mybir.MemoryLocationSet):
                continue
            name = alloc.memorylocations[0].name
            if alloc.kind == "ExternalInput":
                if name != pname:
                    in_names.append(name)
            elif alloc.kind == "ExternalOutput":
                out_names.append(name)
                shape = tuple(alloc.tensor_shape)
                dtype = mybir.dt.np(alloc.dtype)
                out_avals.append(jax.core.ShapedArray(shape, dtype))
                zero_shapes.append((shape, dtype))
        self.in_names, self.out_names, self.out_avals = in_names, out_names, out_avals
        all_in = list(in_names) + list(out_names)
        if pname is not None:
            all_in.append(pname)
        n_params, n_outs = len(in_names), len(out_names)

        def _body(*args):
            operands = list(args)
            if pname is not None:
                operands.append(bass2jax.partition_id_tensor())
            return tuple(_bass_exec_p.bind(
                *operands, out_avals=tuple(out_avals), in_names=tuple(all_in),
                out_names=tuple(out_names), lowering_input_output_aliases=(),
                sim_require_finite=True, sim_require_nnan=True, nc=nc))

        devices = jax.devices()[:NC]
        self.mesh = Mesh(np.asarray(devices), ("core",))
        specs_in = (PartitionSpec("core"),) * (n_params + n_outs)
        specs_out = (PartitionSpec("core"),) * n_outs
        self.fn = jax.jit(
            shard_map(_body, mesh=self.mesh, in_specs=specs_in,
                      out_specs=specs_out, check_rep=False),
            keep_unused=True)
        self.sharding = NamedSharding(self.mesh, PartitionSpec("core"))
        self.zeros = [
            self.jax.device_put(
                np.zeros((NC * s[0], *s[1:]), d), self.sharding)
            for s, d in zero_shapes
        ]

    def __call__(self, in_maps):
        concat = [
            np.concatenate([np.asarray(in_maps[c][n]) for c in range(NC)], 0)
            for n in self.in_names
        ]
        args = [self.jax.device_put(a, self.sharding) for a in concat]
        self.jax.block_until_ready(args)
        import time as _time
        _t0 = _time.perf_counter()
        outs = self.fn(*args, *self.zeros)
        self.jax.block_until_ready(outs)
        LAUNCH_TIMES.append(_time.perf_counter() - _t0)
        res = []
        for c in range(NC):
            res.append({
                n: np.asarray(outs[i]).reshape(NC, *self.out_avals[i].shape)[c]
                for i, n in enumerate(self.out_names)
            })
        return res


# ---------------------------------------------------------------- module builder
def _bn_affine(nc, pool, st, g_sb, b_sb, ci):
    """st [32,2] (sum, sumsq over M) -> (a, bb) [32,1] tiles."""
    mean = pool.tile([32, 1], f32, name=f"bn_mean{ci}")
    ex2 = pool.tile([32, 1], f32, name=f"bn_ex2{ci}")
    nc.vector.tensor_scalar_mul(mean[:], st[:, 0:1], 1.0 / M)
    nc.vector.tensor_scalar_mul(ex2[:], st[:, 1:2], 1.0 / M)
    m2 = pool.tile([32, 1], f32, name=f"bn_m2{ci}")
    nc.vector.tensor_tensor(out=m2[:], in0=mean[:], in1=mean[:], op=ALU.mult)
    var = pool.tile([32, 1], f32, name=f"bn_var{ci}")
    nc.vector.tensor_tensor(out=var[:], in0=ex2[:], in1=m2[:], op=ALU.subtract)
    vp = pool.tile([32, 1], f32, name=f"bn_vp{ci}")
    nc.vector.tensor_scalar_add(vp[:], var[:], EPS)
    std = pool.tile([32, 1], f32, name=f"bn_std{ci}")
    nc.scalar.activation(out=std[:], in_=vp[:], func=ACT.Sqrt)
    inv = pool.tile([32, 1], f32, name=f"bn_inv{ci}")
    nc.vector.reciprocal(inv[:], std[:])
    a = pool.tile([32, 1], f32, name=f"bn_a{ci}")
    nc.vector.tensor_tensor(out=a[:], in0=g_sb[:], in1=inv[:], op=ALU.mult)
    ma = pool.tile([32, 1], f32, name=f"bn_ma{ci}")
    nc.vector.tensor_tensor(out=ma[:], in0=mean[:], in1=a[:], op=ALU.mult)
    bb = pool.tile([32, 1], f32, name=f"bn_bb{ci}")
    nc.vector.tensor_tensor(out=bb[:], in0=b_sb[:], in1=ma[:], op=ALU.subtract)
    return a, bb


DEBUG = False
DEBUG_RESULTS = None
PR = 64
_gq = [0]


def _gather(nc, out_ap, table_ap, idx_col, nq=4):
    """One indirect DMA: 128 rows (one index per partition) — the only
    batching the SWDGE ucode supports (one index per partition per instr)."""
    inst = nc.gpsimd.indirect_dma_start(
        out=out_ap, out_offset=None, in_=table_ap,
        in_offset=bass.IndirectOffsetOnAxis(ap=idx_col, axis=0))
    q = _gq[0] % nq
    _gq[0] += 1
    if q:
        inst.ins.queue = f"qPoolDynamic{q}"


def build_fused(dmax, debug=False):
    nc = bass.Bass(num_swdge_queues=4)
    # ---- parameters (per core)
    pf = nc.declare_dram_parameter("pf", [N + dmax, CIN], f32, isOutput=False)
    vstart = nc.declare_dram_parameter("vstart", [MsP, 1], i32, isOutput=False)
    vmask = nc.declare_dram_parameter("vmask", [MsP, dmax], f32, isOutput=False)
    rcp = nc.declare_dram_parameter("rcp", [MsP, 1], f32, isOutput=False)
    smat = nc.declare_dram_parameter("smat", [dmax * CIN, CIN], f32, isOutput=False)
    nbrs = nc.declare_dram_parameter("nbrs", [MsP, K], i32, isOutput=False)
    didx = nc.declare_dram_parameter("didx", [NpP, KD], i32, isOutput=False)
    wdev = nc.declare_dram_parameter("wdev", [NpP, KD], f32, isOutput=False)
    w1 = nc.declare_dram_parameter("w1", [128, C0], f32, isOutput=False)
    w2 = nc.declare_dram_parameter("w2", [896, C0], bf16, isOutput=False)
    wr1 = nc.declare_dram_parameter("wr1", [896, C0], bf16, isOutput=False)
    wr2 = nc.declare_dram_parameter("wr2", [896, C0], bf16, isOutput=False)
    gps = [nc.declare_dram_parameter(f"gp{i}", [C0], f32, isOutput=False)
           for i in range(4)]
    bps = [nc.declare_dram_parameter(f"bp{i}", [C0], f32, isOutput=False)
           for i in range(4)]
    wc = nc.declare_dram_parameter("wc", [C0, C0], f32, isOutput=False)
    bc = nc.declare_dram_parameter("bc", [1, C0], f32, isOutput=False)
    out = nc.declare_dram_parameter("out", [NpP, NCLS], f32, isOutput=True)
    dbg = {}
    if debug:
        for nm, w in [("vox", CIN), ("h1", C0), ("h2", C0), ("r1", C0),
                      ("y", C0)]:
            dbg[nm] = nc.declare_dram_parameter(f"dbg_{nm}", [NC * PR, w], f32,
                                                isOutput=True)
        dbg["rawT0"] = nc.declare_dram_parameter("dbg_rawT0", [32, PR], f32,
                                                 isOutput=True)

    # ---- internal DRAM
    vox_sh = nc.dram_tensor("vox_sh", [MsP, CIN], f32)
    vox_full = nc.dram_tensor("vox_full", [MT, CIN], f32, addr_space="Shared")
    h_sh = [nc.dram_tensor(f"h_sh{i}", [MsP, C0], bf16) for i in range(4)]
    h_full = [nc.dram_tensor(f"h_full{i}", [MT, C0], bf16, addr_space="Shared")
              for i in range(4)]
    rawT = [nc.dram_tensor(f"rawT{i}", [32, MsP], f32) for i in range(4)]
    st_in = [nc.dram_tensor(f"st_in{i}", [32, 2], f32) for i in range(4)]
    st_out = [nc.dram_tensor(f"st_out{i}", [32, 2], f32, addr_space="Shared")
              for i in range(4)]

    conv_ws = [w1, w2, wr1, wr2]
    conv_cin = [CIN, C0, C0, C0]
    conv_tab = [vox_full, h_full[0], h_full[1], h_full[2]]

    with TileContext(nc) as tc:
        with tc.tile_pool(name="const", bufs=1) as cp:
            ident = cp.tile([128, 128], f32, name="ident")
            make_identity(nc, ident[:])
            identb = cp.tile([128, 128], bf16, name="identb")
            make_identity(nc, identb[:])
            zt = cp.tile([97, 128], bf16, name="zt")
            nc.vector.memset(zt[:], 0.0)
            zb = cp.tile([128, 128], bf16, name="zb")
            nc.vector.memset(zb[:], 0.0)

            # ================= stage 1: voxelize =================
            GWv = dmax * CIN
            with (
                tc.tile_pool(name="sbV", bufs=3) as sb,
                tc.tile_pool(name="ppV", bufs=2, space="PSUM") as pp,
            ):
                ssb = cp.tile([GWv, CIN], f32, name="ssb")
                nc.sync.dma_start(out=ssb[:], in_=smat[:])
                vst_r = vstart[:].rearrange("(s t p) o -> s p t o", t=SUP, p=128)
                vmk_r = vmask[:].rearrange("(s t p) k -> s p t k", t=SUP, p=128)
                rcp_r = rcp[:].rearrange("(s t p) o -> s p t o", t=SUP, p=128)
                vout_r = vox_sh[:].rearrange("(s t p) c -> s p t c", t=SUP, p=128)
                for s in range(NSUP_V):
                    idx = sb.tile([128, SUP], i32, name="idxV", tag="idxV")
                    nc.sync.dma_start(
                        out=idx[:].rearrange("p (t o) -> p t o", t=SUP),
                        in_=vst_r[s])
                    # points are sorted by voxel: one indexed contiguous run of
                    # dmax point rows per voxel (one gather per 128 voxels)
                    G = sb.tile([128, SUP * GWv], f32, name="GV", tag="GV")
                    for t in range(SUP):
                        _gather(nc, G[:, t * GWv:(t + 1) * GWv],
                                pf[:], idx[:, t:t + 1])
                    mk = sb.tile([128, SUP * dmax], f32, name="mkV", tag="mkV")
                    nc.sync.dma_start(
                        out=mk[:].rearrange("p (t k) -> p t k", t=SUP),
                        in_=vmk_r[s])
                    Gm = sb.tile([128, SUP * GWv], f32, name="GmV", tag="GmV")
                    mkv = mk[:].rearrange("p (t d) -> p t d", t=SUP)
                    mkb = bass.AP(mkv.tensor, mkv.offset,
                                  [list(mkv.ap[0]), list(mkv.ap[1]),
                                   list(mkv.ap[2]), [0, CIN]])
                    nc.vector.tensor_tensor(
                        out=Gm[:].rearrange("p (t d c) -> p t d c", t=SUP, d=dmax),
                        in0=G[:].rearrange("p (t d c) -> p t d c", t=SUP, d=dmax),
                        in1=mkb, op=ALU.mult)
                    pgt = pp.tile([128, 512], f32, name="pgtV", tag="pgtV")
                    for t in range(SUP):
                        nc.tensor.transpose(out=pgt[:GWv, t * 128:(t + 1) * 128],
                                            in_=Gm[:, t * GWv:(t + 1) * GWv],
                                            identity=ident[:])
                    GT = sb.tile([128, 512], f32, name="GTV", tag="GTV")
                    nc.vector.tensor_copy(out=GT[:GWv, :], in_=pgt[:GWv, :])
                    pv = pp.tile([128, SUP * CIN], f32, name="pvV", tag="pvV")
                    for t in range(SUP):
                        nc.tensor.matmul(out=pv[:, t * CIN:(t + 1) * CIN],
                                         lhsT=GT[:GWv, t * 128:(t + 1) * 128],
                                         rhs=ssb[:], start=True, stop=True)
                    rc = sb.tile([128, SUP], f32, name="rcV", tag="rcV")
                    nc.sync.dma_start(
                        out=rc[:].rearrange("p (t o) -> p t o", t=SUP),
                        in_=rcp_r[s])
                    vsb = sb.tile([128, SUP * CIN], f32, name="vsbV", tag="vsbV")
                    rcb = bass.AP(rc[:].tensor, rc[:].offset,
                                  [list(rc[:].ap[0]), [1, SUP], [0, CIN]])
                    nc.vector.tensor_tensor(
                        out=vsb[:].rearrange("p (t c) -> p t c", t=SUP),
                        in0=pv[:].rearrange("p (t c) -> p t c", t=SUP),
                        in1=rcb, op=ALU.mult)
                    nc.sync.dma_start(out=vout_r[s],
                                      in_=vsb[:].rearrange("p (t c) -> p t c",
                                                           t=SUP))
            nc.gpsimd.collective_compute("AllGather", ALU.bypass, RG,
                                         ins=[vox_sh[:]], outs=[vox_full[:]])

            # ================= stages 2-5: conv layers =================
            nbrs_r = nbrs[:].rearrange("(s t p) k -> s p t k", t=SUP, p=128)
            for ci in range(4):
                cc = conv_cin[ci]
                GW = K * cc
                nchunk = (GW + 127) // 128
                table = conv_tab[ci]
                residual = (ci == 3)
                tdt = f32 if ci == 0 else bf16      # gather-path dtype
                tid = ident if ci == 0 else identb
                # ---- pass A: raw conv -> rawT + stats
                with (
                    tc.tile_pool(name=f"sbA{ci}", bufs=3) as sb,
                    tc.tile_pool(name=f"ppA{ci}", bufs=2, space="PSUM") as pp,
                ):
                    wsb = cp.tile([128, nchunk * C0], tdt, name=f"wsb{ci}")
                    nc.sync.dma_start(
                        out=wsb[:].rearrange("p (j c) -> p j c", j=nchunk),
                        in_=conv_ws[ci][:].rearrange("(j p) c -> p j c", p=128))
                    sums = cp.tile([32, NSUP_V], f32, name=f"sums{ci}")
                    sqs = cp.tile([32, NSUP_V], f32, name=f"sqs{ci}")
                    for s in range(NSUP_V):
                        idx = sb.tile([128, SUP * K], i32, name="idxA", tag="idxA")
                        nc.sync.dma_start(
                            out=idx[:].rearrange("p (t k) -> p t k", t=SUP),
                            in_=nbrs_r[s])
                        G = sb.tile([128, SUP * GW], tdt, name="GA", tag="GA")
                        for t in range(SUP):
                            for k in range(K):
                                _gather(nc,
                                        G[:, t * GW + k * cc: t * GW + (k + 1) * cc],
                                        table[:], idx[:, t * K + k: t * K + k + 1])
                        po = pp.tile([32, 512], f32, name="poA", tag="poA")
                        for j in range(nchunk):
                            pgt = pp.tile([128, 512], tdt, name="pgtA", tag="pgtA")
                            cw = min(128, GW - j * 128)
                            if cw < 128:
                                if tdt == f32:
                                    nc.vector.memset(pgt[:], 0.0)
                                else:
                                    # DVE can't memset bf16 PSUM; zero via PE
                                    for t in range(SUP):
                                        nc.tensor.transpose(
                                            out=pgt[:, t * 128:(t + 1) * 128],
                                            in_=zb[:], identity=identb[:])
                            for t in range(SUP):
                                nc.tensor.transpose(
                                    out=pgt[:cw, t * 128:(t + 1) * 128],
                                    in_=G[:, t * GW + j * 128: t * GW + j * 128 + cw],
                                    identity=tid[:])
                            GT = sb.tile([128, 512], tdt, name="GTA", tag="GTA")
                            if j % 2 == 0:
                                nc.vector.tensor_copy(out=GT[:], in_=pgt[:])
                            else:
                                nc.scalar.activation(out=GT[:], in_=pgt[:],
                                                     func=ACT.Copy)
                            nc.tensor.matmul(out=po[:],
                                             lhsT=wsb[:, j * C0:(j + 1) * C0],
                                             rhs=GT[:], start=(j == 0),
                                             stop=(j == nchunk - 1))
                        rawsb = sb.tile([32, 512], f32, name="rawA", tag="rawA")
                        nc.scalar.activation(out=rawsb[:], in_=po[:], func=ACT.Copy,
                                             accum_out=sums[:, s:s + 1])
                        sqsb = sb.tile([32, 512], f32, name="sqA", tag="sqA")
                        nc.vector.tensor_tensor(out=sqsb[:], in0=rawsb[:],
                                                in1=rawsb[:], op=ALU.mult)
                        nc.vector.tensor_reduce(out=sqs[:, s:s + 1], in_=sqsb[:],
                                                axis=mybir.AxisListType.X,
                                                op=ALU.add)
                        nc.sync.dma_start(out=rawT[ci][:, s * 512:(s + 1) * 512],
                                          in_=rawsb[:])
                    stats = cp.tile([32, 2], f32, name=f"stats{ci}")
                    nc.vector.tensor_reduce(out=stats[:, 0:1], in_=sums[:],
                                            axis=mybir.AxisListType.X, op=ALU.add)
                    nc.vector.tensor_reduce(out=stats[:, 1:2], in_=sqs[:],
                                            axis=mybir.AxisListType.X, op=ALU.add)
                    nc.sync.dma_start(out=st_in[ci][:], in_=stats[:])
                    nc.gpsimd.collective_compute("AllReduce", ALU.add, RG,
                                                 ins=[st_in[ci][:]],
                                                 outs=[st_out[ci][:]])
                    star = cp.tile([32, 2], f32, name=f"star{ci}")
                    nc.sync.dma_start(out=star[:], in_=st_out[ci][:])
                    gsb = cp.tile([32, 1], f32, name=f"gsb{ci}")
                    bsb = cp.tile([32, 1], f32, name=f"bsb{ci}")
                    nc.sync.dma_start(out=gsb[:], in_=gps[ci][:, None])
                    nc.sync.dma_start(out=bsb[:], in_=bps[ci][:, None])
                    a, bb = _bn_affine(nc, cp, star, gsb, bsb, ci)

                # ---- pass B: BN affine (+ residual / classifier) -> h_sh[ci]
                hout = h_sh[ci]
                with (
                    tc.tile_pool(name=f"sbB{ci}", bufs=3) as sb,
                    tc.tile_pool(name=f"ppB{ci}", bufs=2, space="PSUM") as pp,
                ):
                    if residual:
                        wcsb = cp.tile([C0, C0], f32, name="wcsb")
                        nc.sync.dma_start(out=wcsb[:], in_=wc[:])
                        h2_r = h_sh[1][:].rearrange("(s t p) c -> s p t c",
                                                    t=SUP, p=128)
                    hout_r = hout[:].rearrange("(s t p) c -> s p t c", t=SUP, p=128)
                    for s in range(NSUP_V):
                        raw2 = sb.tile([32, 512], f32, name="raw2", tag="raw2")
                        nc.sync.dma_start(out=raw2[:],
                                          in_=rawT[ci][:, s * 512:(s + 1) * 512])
                        if not residual:
                            hT = sb.tile([32, 512], f32, name="hT", tag="hT")
                            nc.scalar.activation(out=hT[:], in_=raw2[:],
                                                 func=ACT.Relu, bias=bb[:],
                                                 scale=a[:])
                            ph = pp.tile([128, 128], f32, name="ph", tag="ph")
                            for t in range(SUP):
                                nc.tensor.transpose(
                                    out=ph[:, t * C0:(t + 1) * C0],
                                    in_=hT[:, t * 128:(t + 1) * 128],
                                    identity=ident[:32, :32])
                            hsb = sb.tile([128, 128], bf16, name="hsb", tag="hsb")
                            nc.vector.tensor_copy(out=hsb[:], in_=ph[:])
                            nc.sync.dma_start(
                                out=hout_r[s],
                                in_=hsb[:].rearrange("p (t c) -> p t c", t=SUP))
                        else:
                            t0 = sb.tile([32, 512], f32, name="t0", tag="t0")
                            nc.scalar.activation(out=t0[:], in_=raw2[:],
                                                 func=ACT.Identity, bias=bb[:],
                                                 scale=a[:])
                            h2sb = sb.tile([128, 128], f32, name="h2sb", tag="h2sb")
                            nc.gpsimd.dma_start(
                                out=h2sb[:].rearrange("p (t c) -> p t c", t=SUP),
                                in_=h2_r[s])
                            ph2 = pp.tile([32, 512], f32, name="ph2", tag="ph2")
                            for t in range(SUP):
                                nc.tensor.transpose(
                                    out=ph2[:, t * 128:(t + 1) * 128],
                                    in_=h2sb[:, t * C0:(t + 1) * C0],
                                    identity=ident[:])
                            s1 = sb.tile([32, 512], f32, name="s1", tag="s1")
                            nc.vector.tensor_tensor(out=s1[:], in0=t0[:],
                                                    in1=ph2[:], op=ALU.add)
                            h3 = sb.tile([32, 512], f32, name="h3", tag="h3")
                            nc.vector.tensor_scalar_max(h3[:], s1[:], 0.0)
                            py = pp.tile([128, 128], f32, name="py", tag="py")
                            for t in range(SUP):
                                nc.tensor.matmul(
                                    out=py[:, t * C0:(t + 1) * C0],
                                    lhsT=h3[:, t * 128:(t + 1) * 128],
                                    rhs=wcsb[:], start=True, stop=True)
                            ysb = sb.tile([128, 128], bf16, name="ysb", tag="ysb")
                            nc.vector.tensor_copy(out=ysb[:], in_=py[:])
                            nc.sync.dma_start(
                                out=hout_r[s],
                                in_=ysb[:].rearrange("p (t c) -> p t c", t=SUP))
                    # zero the shard's pad rows (Ms..MsP) so gathers of pad
                    # indices and the ZR row read zeros
                    nc.sync.dma_start(
                        out=hout[Ms:MsP].rearrange("(p r) c -> p r c", p=97),
                        in_=zt[:].rearrange("p (r c) -> p r c", r=4))
                nc.gpsimd.collective_compute("AllGather", ALU.bypass, RG,
                                             ins=[hout[:]], outs=[h_full[ci][:]])

            # ================= stage 6: devoxelize =================
            ytab = h_full[3]
            with (
                tc.tile_pool(name="sbD", bufs=3) as sb,
                tc.tile_pool(name="ppD", bufs=2, space="PSUM") as pp,
            ):
                ones = cp.tile([1, 128], f32, name="onesD")
                nc.gpsimd.memset(ones[:], 1.0)
                bcs = cp.tile([1, C0], f32, name="bcsD")
                nc.sync.dma_start(out=bcs[:], in_=bc[:])
                pbc = pp.tile([128, C0], f32, name="pbc")
                nc.tensor.matmul(out=pbc[:], lhsT=ones[:], rhs=bcs[:],
                                 start=True, stop=True)
                bcb = cp.tile([128, C0], f32, name="bcbD")
                nc.vector.tensor_copy(out=bcb[:], in_=pbc[:])

                didx_r = didx[:].rearrange("(s t p) k -> s p t k", t=SUP, p=128)
                wdev_r = wdev[:].rearrange("(s t p) k -> s p t k", t=SUP, p=128)
                out_r = out[:].rearrange("(s t p) c -> s p t c", t=SUP, p=128)
                GW = KD * C0
                for s in range(NSUP_P):
                    idx = sb.tile([128, SUP * KD], i32, name="idxD", tag="idxD")
                    nc.sync.dma_start(
                        out=idx[:].rearrange("p (t k) -> p t k", t=SUP),
                        in_=didx_r[s])

                    G = sb.tile([128, SUP * GW], bf16, name="GD", tag="GD")
                    for t in range(SUP):
                        for k in range(KD):
                            _gather(nc,
                                    G[:, t * GW + k * C0: t * GW + (k + 1) * C0],
                                    ytab[:], idx[:, t * KD + k: t * KD + k + 1])
                    Gf = sb.tile([128, SUP * GW], f32, name="GfD", tag="GfD")
                    nc.scalar.activation(out=Gf[:], in_=G[:], func=ACT.Copy)
                    w4 = sb.tile([128, SUP * KD], f32, name="w4", tag="w4")
                    nc.sync.dma_start(
                        out=w4[:].rearrange("p (t k) -> p t k", t=SUP),
                        in_=wdev_r[s])
                    prod = sb.tile([128, SUP * GW], f32, name="prod", tag="prod")
                    gv = Gf[:].rearrange("p (t k c) -> p t k c", t=SUP, k=KD, c=C0)
                    pvw = prod[:].rearrange("p (t c k) -> p t k c", t=SUP, c=C0,
                                            k=KD)
                    wv = w4[:].rearrange("p (t k) -> p t k", t=SUP)
                    wb = bass.AP(wv.tensor, wv.offset,
                                 [list(wv.ap[0]), list(wv.ap[1]), list(wv.ap[2]),
                                  [0, C0]])
                    nc.vector.tensor_tensor(out=pvw, in0=gv, in1=wb, op=ALU.mult)
                    pts = sb.tile([128, SUP * C0], f32, name="pts", tag="pts")
                    nc.vector.tensor_reduce(
                        out=pts[:].rearrange("p (t c) -> p t c", t=SUP),
                        in_=prod[:].rearrange("p (t c k) -> p t c k", t=SUP,
                                              c=C0, k=KD),
                        axis=mybir.AxisListType.X, op=ALU.add)
                    res = sb.tile([128, SUP * C0], f32, name="res", tag="res")
                    bcv = bass.AP(bcb[:].tensor, bcb[:].offset,
                                  [list(bcb[:].ap[0]), [0, SUP],
                                   list(bcb[:].ap[1])])
                    nc.vector.tensor_tensor(
                        out=res[:].rearrange("p (t c) -> p t c", t=SUP),
                        in0=pts[:].rearrange("p (t c) -> p t c", t=SUP),
                        in1=bcv, op=ALU.add)
                    nc.sync.dma_start(
                        out=out_r[s],
                        in_=res[:].rearrange("p (t c) -> p t c", t=SUP)[:, :, :NCLS])

            if debug:
                with tc.tile_pool(name="sbDbg", bufs=2) as sb:
                    tabs = dict(vox=vox_full, h1=h_full[0], h2=h_full[1],
                                r1=h_full[2], y=h_full[3])
                    for nm, tab in tabs.items():
                        w = CIN if nm == "vox" else C0
                        for r in range(NC):
                            t = sb.tile([PR, w], f32, name=f"dbg_{nm}{r}",
                                        tag="dbgt")
                            nc.sync.dma_start(
                                out=t[:], in_=tab[r * MsP:r * MsP + PR])
                            nc.sync.dma_start(
                                out=dbg[nm][r * PR:(r + 1) * PR], in_=t[:])
                    t = sb.tile([32, PR], f32, name="dbg_rawT0", tag="dbgr")
                    nc.sync.dma_start(out=t[:], in_=rawT[0][:, :PR])
                    nc.sync.dma_start(out=dbg["rawT0"][:], in_=t[:])
    return _install_waitfix(nc)


# ---------------------------------------------------------------- host side
def _remap(g):
    g = np.asarray(g)
    gc = np.clip(g, 0, M - 1)
    s = gc // Ms
    out = s * MsP + (gc - s * Ms)
    return np.where(g < 0, ZR, out).astype(np.int32)


def _stack_w(Wk, cols):
    """W [27, cin, 32] -> padded [nchunk*128, 32] stack over (k, cin)."""
    Wk = np.asarray(Wk, np.float32)
    kcin = Wk.shape[0] * Wk.shape[1]
    nchunk = (27 * Wk.shape[1] + 127) // 128
    o = np.zeros((nchunk * 128, C0), np.float32)
    o[:kcin] = Wk.reshape(kcin, C0)
    return o


def _get_runner(dmax):
    key = ("fused", dmax, DEBUG)
    if key not in _cache:
        _cache[key] = _Runner(build_fused(dmax, debug=DEBUG))
    return _cache[key]


def kernel(point_fea, idx_query, nbrs, idx_dev, w_dev,
           W_s1, W_s2, g_s1, b_s1, g_s2, b_s2,
           W_r1, W_r2, g_r1, b_r1, g_r2, b_r2, W_c, b_c):
    point_fea = np.asarray(point_fea, np.float32)
    idx_query = np.asarray(idx_query, np.int32)
    nbrs = np.asarray(nbrs, np.int32)
    idx_dev = np.asarray(idx_dev, np.int32)
    w_dev = np.asarray(w_dev, np.float32)

    # ---- host preprocessing (index plumbing only)
    counts = np.bincount(idx_query, minlength=M)
    dmax = int(counts.max())
    order = np.argsort(idx_query, kind="stable")
    starts = np.zeros(M + 1, np.int64)
    np.cumsum(counts, out=starts[1:])
    # points sorted by voxel id: each voxel's points are contiguous rows
    pf_table = np.zeros((N + dmax, CIN), np.float32)
    pf_table[:N] = point_fea[order]
    vstart_full = starts[:M].astype(np.int32)          # [M]
    vmask_full = (np.arange(dmax)[None, :]
                  < counts[:, None]).astype(np.float32)  # [M, dmax]
    recip_full = (1.0 / np.maximum(counts, 1)).astype(np.float32)

    smat = np.zeros((dmax * CIN, CIN), np.float32)
    for d in range(dmax):
        smat[d * CIN:(d + 1) * CIN] = np.eye(CIN, dtype=np.float32)

    nb_remap = _remap(nbrs)                     # [M, 27]
    per = []
    for c in range(NC):
        vs = slice(c * Ms, (c + 1) * Ms)
        ps = slice(c * Np, (c + 1) * Np)
        vstart = np.full((MsP, 1), N, np.int32)
        vstart[:Ms, 0] = vstart_full[vs]
        vmask = np.zeros((MsP, dmax), np.float32)
        vmask[:Ms] = vmask_full[vs]
        rcp = np.zeros((MsP, 1), np.float32)
        rcp[:Ms, 0] = recip_full[vs]
        nb28 = np.full((MsP, K), ZR, np.int32)
        nb28[:Ms] = nb_remap[vs]
        didx = np.full((NpP, KD), ZR, np.int32)
        didx[:Np] = _remap(idx_dev[ps])
        wd = np.zeros((NpP, KD), np.float32)
        wd[:Np] = w_dev[ps]
        per.append(dict(vstart=vstart, vmask=vmask, rcp=rcp, nb28=nb28,
                        didx=didx, wd=wd))

    import ml_dtypes
    BF = ml_dtypes.bfloat16
    W1s = _stack_w(np.asarray(W_s1), CIN)
    W2s = _stack_w(np.asarray(W_s2), C0).astype(BF)
    Wr1s = _stack_w(np.asarray(W_r1), C0).astype(BF)
    Wr2s = _stack_w(np.asarray(W_r2), C0).astype(BF)
    Wc_pad = np.zeros((C0, C0), np.float32)
    Wc_pad[:, :NCLS] = np.asarray(W_c)
    bc_pad = np.zeros((1, C0), np.float32)
    bc_pad[0, :NCLS] = np.asarray(b_c)

    R = _get_runner(dmax)
    res = R([dict(pf=pf_table, vstart=per[c]["vstart"], vmask=per[c]["vmask"],
                  rcp=per[c]["rcp"],
                  smat=smat, nbrs=per[c]["nb28"], didx=per[c]["didx"],
                  wdev=per[c]["wd"], w1=W1s, w2=W2s, wr1=Wr1s, wr2=Wr2s,
                  gp0=np.asarray(g_s1, np.float32),
                  bp0=np.asarray(b_s1, np.float32),
                  gp1=np.asarray(g_s2, np.float32),
                  bp1=np.asarray(b_s2, np.float32),
                  gp2=np.asarray(g_r1, np.float32),
                  bp2=np.asarray(b_r1, np.float32),
                  gp3=np.asarray(g_r2, np.float32),
                  bp3=np.asarray(b_r2, np.float32),
                  wc=Wc_pad, bc=bc_pad)
             for c in range(NC)])
    if DEBUG:
        global DEBUG_RESULTS
        DEBUG_RESULTS = res
    out = np.concatenate([res[c]["out"][:Np] for c in range(NC)], 0)
    return np.ascontiguousarray(out)
